# revision 71
# baseline (speedup 1.0000x reference)
"""Causal MHA kernel for 8 TRN2 NeuronCores.

Problem: x[4,2048,1024], 16 heads, hd=64, causal softmax attention, f32.

Sharding: core c handles batch c%4 and head-half c//4 (8 heads).
Each core computes its 8 heads' attention plus the row-slice of the
output projection; the host sums the two partials per batch (the
all-reduce of the row-parallel W_o split) and adds b_o.

v5: bf16 operands everywhere (PSUM accumulation stays f32; operand
quantization ~0.5% rel err, gate 2e-2). Weights are DMA'd once and
stay resident in SBUF.

Schedule: one software-pipelined stream over rep x token-chunk. Score
tiles are pipelined singly (one 128x512 tile per stage) with a 2-tile
lookahead through a 4-buffer PSUM pool — deeper and finer than pair
granularity, which only left ~44ns of per-slot slack against ScalarE's
exp cadence. During attention for query chunk t (paced by ScalarE exp
throughput), the PE work of the NEXT chunk's Q/K/V projections and the
PREVIOUS chunk's output projection is drained in at tile granularity,
so the in-order PE queue always has independent work while ScalarE
drains.
The chunk stream crosses rep boundaries; Q^T/K^T/V^T live in two
SBUF bank sets (rep parity) so the next rep's projections can overlap
the previous rep's last attention chunk without WAR stalls. ScalarE
runs ONLY exp; every PSUM->SBUF copy is on the Vector engine; memsets
and DMA dispatch are on GpSimd/SP.

Attention uses a flipped ctx product: probs tiles [keys, q] are the
stationary operand and V_ext [keys, hd+1] the moving one, producing
ctx in [q, hd] orientation with full PE output-column utilization
(half the streaming cycles of the [hd+1, q] orientation) and natural
causal skipping of all-masked 128-blocks. The trailing ones-column of
V_ext makes PSUM col 64 of each q-block the softmax denominator, so
normalization is a per-partition reciprocal + scale on DVE (no PE
broadcast). A PE transpose per q-block returns ctx to the f-major
layout the W_o projection needs. Within a head, score matmuls run
2 tiles ahead of the ctx matmuls; a head's ctx transposes are
deferred into the next head's stream (the last head's across the
chunk boundary). Diagonal 128x512 score tiles are
trimmed: exp only the valid column range and one [128,128] triangular
mask multiply; below-diagonal prob columns are never read at all in
the flipped ctx, so they need no zeroing.

Device-side layout: everything transposed. Host ships x[b].T so the
contraction dim (D) lands on SBUF partitions.
"""

import numpy as np
import ml_dtypes

BF16 = ml_dtypes.bfloat16

B, S, D, H, HD = 4, 2048, 1024, 16, 64
HL = 8            # heads per core
F = HL * HD       # 512 local head features
P = 128
CH = 512          # free-dim chunk for matmuls
NKT = D // P      # 8 contraction tiles for projections
NMT = F // P      # 4 head-pair tiles
NCH = S // CH     # 4 token chunks
NKA = S // P      # 16 attention key tiles

_NC_CACHE = {}


def _build_nc(reps=1):
    from contextlib import ExitStack

    import concourse.bass as bass
    import concourse.tile as tile
    from concourse import bacc, mybir
    from concourse.masks import make_identity

    f32 = mybir.dt.float32
    bf16 = mybir.dt.bfloat16
    AF = mybir.ActivationFunctionType
    ALU = mybir.AluOpType

    nc = bacc.Bacc("TRN2", target_bir_lowering=False)
    xt_d = nc.declare_dram_parameter("xt", [D, S], bf16, isOutput=False)
    wq_d = nc.declare_dram_parameter("wq", [D, F], bf16, isOutput=False)
    wk_d = nc.declare_dram_parameter("wk", [D, F], bf16, isOutput=False)
    wv_d = nc.declare_dram_parameter("wv", [D, F], bf16, isOutput=False)
    wo_d = nc.declare_dram_parameter("wo", [F, D], bf16, isOutput=False)
    out_d = nc.declare_dram_parameter("out", [S, D], f32, isOutput=True)
    w_by_name = {"q": wq_d, "k": wk_d, "v": wv_d}

    nbank = 2 if reps > 1 else 1

    with tile.TileContext(nc) as tc, ExitStack() as ctx:
        const_pool = ctx.enter_context(tc.tile_pool(name="const", bufs=1))
        qt_pool = ctx.enter_context(tc.tile_pool(name="qt", bufs=1))
        ve_pool = ctx.enter_context(tc.tile_pool(name="ve", bufs=1))
        wo_pool = ctx.enter_context(tc.tile_pool(name="wo", bufs=1))
        ws_pool = ctx.enter_context(tc.tile_pool(name="ws", bufs=1))
        xt_pool = ctx.enter_context(tc.tile_pool(name="xt", bufs=2))
        vs_pool = ctx.enter_context(tc.tile_pool(name="vstage", bufs=2))
        ptile_pool = ctx.enter_context(tc.tile_pool(name="ptile", bufs=6))
        ctc_pool = ctx.enter_context(tc.tile_pool(name="ctc", bufs=2))
        rec_pool = ctx.enter_context(tc.tile_pool(name="rec", bufs=2))
        ctq_pool = ctx.enter_context(tc.tile_pool(name="ctq", bufs=2))
        osb_pool = ctx.enter_context(tc.tile_pool(name="osb", bufs=2))
        pp_pool = ctx.enter_context(
            tc.tile_pool(name="pp", bufs=2, space="PSUM"))
        ps_s_pool = ctx.enter_context(
            tc.tile_pool(name="ps_s", bufs=4, space="PSUM"))
        pcm_pool = ctx.enter_context(
            tc.tile_pool(name="pcm", bufs=2, space="PSUM"))

        ident = const_pool.tile([P, P], bf16)
        make_identity(nc, ident[:])
        onesf = const_pool.tile([P, 1], bf16)
        nc.vector.memset(onesf[:], 1.0)

        # double-banked Q^T/K^T/V_ext (rep parity) so rep r+1's
        # projections overlap rep r's last attention chunk
        QT, KT, VE4 = [], [], []
        for bk in range(nbank):
            QT.append([qt_pool.tile([P, S], bf16, name=f"qt{bk}_{m}",
                                    tag=f"qt{bk}_{m}")
                       for m in range(NMT)])
            KT.append([qt_pool.tile([P, S], bf16, name=f"kt{bk}_{m}",
                                    tag=f"kt{bk}_{m}")
                       for m in range(NMT)])
            # V_ext: per (head, key-tile) a [128, 65] stationary block;
            # col 64 stays 1.0 (projection copies only touch cols 0..63).
            VE = ve_pool.tile([P, HL * NKA * 65], bf16, name=f"ve{bk}",
                              tag=f"ve{bk}")
            nc.vector.tensor_copy(
                VE[:].rearrange("p (b c) -> p b c", c=65)[:, :, 64:65],
                onesf[:].broadcast_to([P, HL * NKA, 1]),
            )
            VE4.append(VE[:].rearrange("p (h ka c) -> p h ka c",
                                       h=HL, c=65))

        # single [128,128] lower-triangular keep-mask: tri[k,c]=1 iff c>=k
        tri = const_pool.tile([P, P], bf16)
        nc.vector.memset(tri[:], 1.0)
        nc.gpsimd.affine_select(
            out=tri[:],
            in_=tri[:],
            compare_op=ALU.is_ge,
            fill=0.0,
            base=0,
            pattern=[[1, P]],
            channel_multiplier=-1,
        )

        # resident weights, DMA'd in per-128-row slices from the otherwise
        # idle SP sequencer so Pool isn't a dispatch bottleneck.
        WS = {}
        for wname in ("v", "k", "q"):
            ws = ws_pool.tile([P, NKT * F], bf16, name=f"ws_{wname}",
                              tag=f"ws_{wname}")
            for kt in range(NKT):
                nc.sync.dma_start(
                    ws[:, kt * F : (kt + 1) * F],
                    w_by_name[wname][kt * P : (kt + 1) * P, :],
                )
            WS[wname] = ws
        WO = wo_pool.tile([P, NMT * D], bf16)
        for ft in range(NMT):
            nc.sync.dma_start(
                WO[:, ft * D : (ft + 1) * D],
                wo_d[ft * P : (ft + 1) * P, :],
            )

        def dma_xt(tch):
            xt_t = xt_pool.tile([P, NKT * CH], bf16)
            for kt in range(NKT):
                # Pool queue: parallel with SP's weight DMAs
                nc.gpsimd.dma_start(
                    xt_t[:, kt * CH : (kt + 1) * CH],
                    xt_d[kt * P : (kt + 1) * P,
                         tch * CH : (tch + 1) * CH],
                )
            return xt_t

        def proj_half(bk, tch, xt_t, wname, mt, half, state):
            # split per-mt projection into two ~850ns fill units sharing
            # one PSUM accumulator
            ws = WS[wname]
            k0 = half * (NKT // 2)
            if half == 0:
                state["pp"] = pp_pool.tile([P, CH], f32, name="pp",
                                           tag="pp")
            pp = state["pp"]
            for kt in range(k0, k0 + NKT // 2):
                nc.tensor.matmul(
                    pp[:],
                    ws[:, kt * F + mt * P : kt * F + (mt + 1) * P],
                    xt_t[:, kt * CH : (kt + 1) * CH],
                    start=(kt == 0),
                    stop=(kt == NKT - 1),
                )
            if half == 0:
                return
            if wname == "q":
                nc.vector.tensor_copy(
                    QT[bk][mt][:, tch * CH : (tch + 1) * CH], pp[:]
                )
            elif wname == "k":
                nc.vector.tensor_copy(
                    KT[bk][mt][:, tch * CH : (tch + 1) * CH], pp[:]
                )
            else:
                vs = vs_pool.tile([P, CH], bf16)
                nc.vector.tensor_copy(vs[:], pp[:])
                for j in range(CH // P):
                    ka = tch * (CH // P) + j
                    ptp = ps_s_pool.tile([P, P], bf16, tag="ps")
                    nc.tensor.transpose(
                        ptp[:], vs[:, j * P : (j + 1) * P], ident[:]
                    )
                    # both heads' 64-col halves in one copy
                    nc.vector.tensor_copy(
                        VE4[bk][:, 2 * mt : 2 * mt + 2, ka, 0:HD],
                        ptp[:].rearrange(
                            "p (hh c) -> p hh c", hh=2
                        )[:, :, 0:HD],
                    )

        def proj_groups(bk, tch, xt_t):
            out = []
            for w in ("v", "k", "q"):
                for m in range(NMT):
                    st = {}
                    for hf in range(2):
                        out.append(
                            lambda w=w, m=m, hf=hf, st=st:
                                proj_half(bk, tch, xt_t, w, m, hf, st)
                        )
            return out

        def wo_part(qc, ctc, tt4, ncol, state):
            if ncol == 0:
                state["osb"] = osb_pool.tile([P, D], f32, name="osb")
            osb = state["osb"]
            # pp pool, not pcm: pcq buffers are released by the (slower)
            # reciprocal+scale chain, pp buffers by a plain copy
            po = pp_pool.tile([P, CH], f32, name="po", tag="pp")
            for ft in range(NMT):
                nc.tensor.matmul(
                    po[:],
                    ctc[ft][:, tt4 * P : (tt4 + 1) * P],
                    WO[:, ft * D + ncol * CH : ft * D + (ncol + 1) * CH],
                    start=(ft == 0),
                    stop=(ft == NMT - 1),
                )
            # alternate engines: the po->osb copy gates the pp-pool WAR
            # for the next po, and DVE's in-order queue can back up ~3us
            if (tt4 + ncol) % 2 == 0:
                nc.scalar.copy(
                    osb[:, ncol * CH : (ncol + 1) * CH], po[:]
                )
            else:
                nc.vector.tensor_copy(
                    osb[:, ncol * CH : (ncol + 1) * CH], po[:]
                )
            if ncol == D // CH - 1:
                r0 = qc * CH + tt4 * P
                nc.gpsimd.dma_start(out_d[r0 : r0 + P, :], osb[:])

        def wo_groups(qc, ctc):
            out = []
            for t in range(CH // P):
                st = {}
                for ncol in range(D // CH):
                    out.append(
                        lambda t=t, ncol=ncol, st=st:
                            wo_part(qc, ctc, t, ncol, st)
                    )
            return out

        def attention_chunk(bk, qc, fill, carry_tpose=None):
            """Attention for query chunk qc reading bank bk, draining
            `fill` (list of emission callables) at key-pair granularity.
            `carry_tpose` is the previous chunk's last-head ctx transpose
            (emitted here, where its inputs are long ready). Returns
            (ctc, carry) where carry finishes this chunk's last head."""
            nka_q = 4 * qc + 4  # causal: key tiles 0..nka_q-1
            total_slots = HL * nka_q
            fill_state = [0, 0]  # [next fill idx, slot counter]

            def drain_fill():
                idx, slot = fill_state
                while (idx < len(fill)
                       and idx * total_slots <= slot * len(fill)):
                    fill[idx]()
                    idx += 1
                fill_state[0] = idx
                fill_state[1] = slot + 1

            ctc = [ctc_pool.tile([P, CH], bf16, name=f"ctc{m}",
                                 tag=f"ctc{m}")
                   for m in range(NMT)]

            def emit_norm_dve(pcq, ctq):
                # denominators live at col 64 of each qb block ->
                # per-partition scale, no PE broadcast needed.
                rec4 = rec_pool.tile([P, 4], bf16)
                pcq3 = pcq[:].rearrange("p (qb c) -> p qb c", c=HD + 1)
                rec3 = rec4[:].rearrange("p (b o) -> p b o", o=1)
                with nc.allow_low_precision(
                    reason="1/l rounded to bf16 scale"
                ):
                    nc.vector.reciprocal(rec3, pcq3[:, :, HD : HD + 1])
                nc.vector.tensor_mul(
                    ctq[:].rearrange("p (qb c) -> p qb c", c=HD),
                    pcq3[:, :, 0:HD],
                    rec3.broadcast_to([P, 4, HD]),
                )

            def emit_norm_tpose(ctq, mt, hrow):
                # [q, hd] -> [hd, q] for the f-major Wo projection
                for qb in range(4):
                    ptq = ps_s_pool.tile([HD, P], bf16, tag="ps")
                    nc.tensor.transpose(
                        ptq[:], ctq[:, qb * HD : (qb + 1) * HD], ident[:]
                    )
                    nc.vector.tensor_copy(
                        ctc[mt][hrow : hrow + HD, qb * P : (qb + 1) * P],
                        ptq[:],
                    )

            if carry_tpose is not None:
                carry_tpose()
            pending_tpose = None
            for h in range(HL):
                mt = h // 2
                hrow = (h % 2) * HD
                pcq = pcm_pool.tile([P, 4 * (HD + 1)], f32, tag="pcm")
                ctq = ctq_pool.tile([P, 4 * HD], bf16)
                ps1s = {}
                pt1s = {}

                def emit_scores(kt):
                    j = kt - 4 * qc  # diag block index if >= 0
                    lo = j * P if j >= 0 else 0
                    ps1 = ps_s_pool.tile([P, CH], f32, tag="ps")
                    pt1 = ptile_pool.tile([P, CH], bf16)
                    ps1s[kt] = ps1
                    pt1s[kt] = pt1
                    nc.tensor.matmul(
                        ps1[:, lo:],
                        KT[bk][mt][hrow : hrow + HD,
                                   kt * P : (kt + 1) * P],
                        QT[bk][mt][hrow : hrow + HD,
                                   qc * CH + lo : (qc + 1) * CH],
                        start=True,
                        stop=True,
                    )
                    # flipped ctx only reads q-blocks >= j, so the
                    # below-diagonal columns need no zeroing at all
                    nc.scalar.activation(
                        pt1[:, lo:], ps1[:, lo:], AF.Exp, scale=0.125
                    )
                    if j >= 0:
                        nc.vector.tensor_mul(
                            pt1[:, lo : lo + P],
                            pt1[:, lo : lo + P],
                            tri[:],
                        )

                def emit_ctx(kt):
                    pt1 = pt1s.pop(kt)
                    ps1s.pop(kt)
                    j = kt - 4 * qc
                    # masked (triangular) q-block last: its DVE mask
                    # finishes while the other blocks' ctx runs
                    if j >= 0:
                        qb_order = list(range(j + 1, 4)) + [j]
                    else:
                        qb_order = list(range(4))
                    for qb in qb_order:
                        # start=True clears has_written for the WHOLE
                        # bank, so only the very first matmul into the
                        # pcq bank may set it; later qb first-writes
                        # overwrite-where-clear per element.
                        nc.tensor.matmul(
                            pcq[:, qb * (HD + 1)
                                : qb * (HD + 1) + HD + 1],
                            pt1[:, qb * P : (qb + 1) * P],
                            VE4[bk][:, h, kt, :],
                            start=(kt == 0 and qb == qb_order[0]),
                            stop=(kt == min(nka_q - 1, 4 * qc + qb)),
                        )

                # software-pipelined emission with 3-tile lookahead:
                # scores/exp of tiles i+1..i+3 are in flight while ctx
                # of tile i runs, so PE streams while ScalarE drains;
                # the previous head's ctx transposes are emitted a few
                # tiles in.
                LA = 2
                for t in range(min(LA, nka_q)):
                    emit_scores(t)
                for i in range(nka_q):
                    if i + LA < nka_q:
                        emit_scores(i + LA)
                    # fill lands BEFORE this tile's ctx so the PE has
                    # queued work while the tile's exp drains
                    drain_fill()
                    emit_ctx(i)
                    if (i == min(2, nka_q - 1)
                            and pending_tpose is not None):
                        emit_norm_tpose(*pending_tpose)
                        pending_tpose = None
                # the DVE part runs now (frees pcq for the pool); only
                # the PE transposes are deferred.
                emit_norm_dve(pcq, ctq)
                pending_tpose = (ctq, mt, hrow)
            while fill_state[0] < len(fill):
                fill[fill_state[0]]()
                fill_state[0] += 1
            args = pending_tpose
            carry = lambda a=args: emit_norm_tpose(*a)
            return ctc, carry

        # ---- the flattened rep x chunk stream -------------------------
        seq = [(r, t) for r in range(reps) for t in range(NCH)]
        xt_t = dma_xt(0)
        for g in proj_groups(0, 0, xt_t):
            g()
        pending_wo = []
        carry = None
        for idx, (r, tch) in enumerate(seq):
            fill = list(pending_wo)
            pending_wo = []
            if idx + 1 < len(seq):
                nr, nt = seq[idx + 1]
                xt_t = dma_xt(nt)
                fill += proj_groups(nr % nbank, nt, xt_t)
            ctc, carry = attention_chunk(r % nbank, tch, fill, carry)
            pending_wo = wo_groups(tch, ctc)
        carry()
        for g in pending_wo:
            g()

    nc.compile()
    return nc


def _get_nc(reps=1):
    key = f"nc{reps}"
    if key not in _NC_CACHE:
        _NC_CACHE[key] = _build_nc(reps)
    return _NC_CACHE[key]


def _make_in_maps(inputs):
    x = np.asarray(inputs["x"], dtype=np.float32)
    W_q = np.asarray(inputs["W_q"], dtype=np.float32)
    W_k = np.asarray(inputs["W_k"], dtype=np.float32)
    W_v = np.asarray(inputs["W_v"], dtype=np.float32)
    W_o = np.asarray(inputs["W_o"], dtype=np.float32)
    in_maps = []
    for c in range(8):
        b = c % 4
        hh = c // 4
        cols = slice(hh * F, (hh + 1) * F)
        in_maps.append(
            {
                "xt": np.ascontiguousarray(x[b].T).astype(BF16),
                "wq": np.ascontiguousarray(W_q[:, cols]).astype(BF16),
                "wk": np.ascontiguousarray(W_k[:, cols]).astype(BF16),
                "wv": np.ascontiguousarray(W_v[:, cols]).astype(BF16),
                "wo": np.ascontiguousarray(W_o[cols, :]).astype(BF16),
            }
        )
    return in_maps


def kernel(x, W_q, W_k, W_v, W_o, b_o):
    from concourse.bass_utils import run_bass_kernel_spmd

    b_o = np.asarray(b_o, dtype=np.float32)
    nc = _get_nc()
    in_maps = _make_in_maps(
        {"x": x, "W_q": W_q, "W_k": W_k, "W_v": W_v, "W_o": W_o}
    )
    res = run_bass_kernel_spmd(nc, in_maps, core_ids=list(range(8)))

    full = np.empty((B, S, D), dtype=np.float32)
    for b in range(B):
        full[b] = res.results[b]["out"] + res.results[b + 4]["out"] + b_o
    return full


# revision 73
# speedup vs baseline: 1.0409x; 1.0409x over previous
"""Causal MHA kernel for 8 TRN2 NeuronCores.

Problem: x[4,2048,1024], 16 heads, hd=64, causal softmax attention, f32.

Sharding: core c handles batch c%4 and head-half c//4 (8 heads).
Each core computes its 8 heads' attention plus the row-slice of the
output projection; the host sums the two partials per batch (the
all-reduce of the row-parallel W_o split) and adds b_o.

v5: bf16 operands everywhere (PSUM accumulation stays f32; operand
quantization ~0.5% rel err, gate 2e-2). Weights are DMA'd once and
stay resident in SBUF.

Schedule: one software-pipelined stream over rep x token-chunk. Score
tiles are pipelined singly (one 128x512 tile per stage) with a 2-tile
lookahead through a 4-buffer PSUM pool — deeper and finer than pair
granularity, which only left ~44ns of per-slot slack against ScalarE's
exp cadence. During attention for query chunk t (paced by ScalarE exp
throughput), the PE work of the NEXT chunk's Q/K/V projections and the
PREVIOUS chunk's output projection is drained in at tile granularity,
so the in-order PE queue always has independent work while ScalarE
drains.
The chunk stream crosses rep boundaries; Q^T/K^T/V^T live in two
SBUF bank sets (rep parity) so the next rep's projections can overlap
the previous rep's last attention chunk without WAR stalls. ScalarE
runs ONLY exp; every PSUM->SBUF copy is on the Vector engine; memsets
and DMA dispatch are on GpSimd/SP.

Attention uses a flipped ctx product: probs tiles [keys, q] are the
stationary operand and V_ext [keys, hd+1] the moving one, producing
ctx in [q, hd] orientation with full PE output-column utilization
(half the streaming cycles of the [hd+1, q] orientation) and natural
causal skipping of all-masked 128-blocks. The trailing ones-column of
V_ext makes PSUM col 64 of each q-block the softmax denominator, so
normalization is a per-partition reciprocal + scale on DVE (no PE
broadcast). A PE transpose per q-block returns ctx to the f-major
layout the W_o projection needs. Within a head, score matmuls run
2 tiles ahead of the ctx matmuls; a head's ctx transposes are
deferred into the next head's stream (the last head's across the
chunk boundary). Diagonal 128x512 score tiles are
trimmed: exp only the valid column range and one [128,128] triangular
mask multiply; below-diagonal prob columns are never read at all in
the flipped ctx, so they need no zeroing.

Device-side layout: everything transposed. Host ships x[b].T so the
contraction dim (D) lands on SBUF partitions.
"""

import numpy as np
import ml_dtypes

BF16 = ml_dtypes.bfloat16

B, S, D, H, HD = 4, 2048, 1024, 16, 64
HL = 8            # heads per core
F = HL * HD       # 512 local head features
P = 128
CH = 512          # free-dim chunk for matmuls
NKT = D // P      # 8 contraction tiles for projections
NMT = F // P      # 4 head-pair tiles
NCH = S // CH     # 4 token chunks
NKA = S // P      # 16 attention key tiles

_NC_CACHE = {}


def _build_nc(reps=1):
    from contextlib import ExitStack

    import concourse.bass as bass
    import concourse.tile as tile
    from concourse import bacc, mybir
    from concourse.masks import make_identity

    f32 = mybir.dt.float32
    bf16 = mybir.dt.bfloat16
    AF = mybir.ActivationFunctionType
    ALU = mybir.AluOpType

    nc = bacc.Bacc("TRN2", target_bir_lowering=False)
    xt_d = nc.declare_dram_parameter("xt", [D, S], bf16, isOutput=False)
    wq_d = nc.declare_dram_parameter("wq", [D, F], bf16, isOutput=False)
    wk_d = nc.declare_dram_parameter("wk", [D, F], bf16, isOutput=False)
    wv_d = nc.declare_dram_parameter("wv", [D, F], bf16, isOutput=False)
    wo_d = nc.declare_dram_parameter("wo", [F, D], bf16, isOutput=False)
    out_d = nc.declare_dram_parameter("out", [S, D], f32, isOutput=True)
    w_by_name = {"q": wq_d, "k": wk_d, "v": wv_d}

    nbank = 2 if reps > 1 else 1

    with tile.TileContext(nc) as tc, ExitStack() as ctx:
        const_pool = ctx.enter_context(tc.tile_pool(name="const", bufs=1))
        qt_pool = ctx.enter_context(tc.tile_pool(name="qt", bufs=1))
        ve_pool = ctx.enter_context(tc.tile_pool(name="ve", bufs=1))
        wo_pool = ctx.enter_context(tc.tile_pool(name="wo", bufs=1))
        ws_pool = ctx.enter_context(tc.tile_pool(name="ws", bufs=1))
        xt_pool = ctx.enter_context(tc.tile_pool(name="xt", bufs=2))
        vs_pool = ctx.enter_context(tc.tile_pool(name="vstage", bufs=2))
        ptile_pool = ctx.enter_context(tc.tile_pool(name="ptile", bufs=6))
        ctc_pool = ctx.enter_context(tc.tile_pool(name="ctc", bufs=2))
        rec_pool = ctx.enter_context(tc.tile_pool(name="rec", bufs=2))
        ctq_pool = ctx.enter_context(tc.tile_pool(name="ctq", bufs=2))
        osb_pool = ctx.enter_context(tc.tile_pool(name="osb", bufs=2))
        pp_pool = ctx.enter_context(
            tc.tile_pool(name="pp", bufs=2, space="PSUM"))
        ps_s_pool = ctx.enter_context(
            tc.tile_pool(name="ps_s", bufs=4, space="PSUM"))
        pcm_pool = ctx.enter_context(
            tc.tile_pool(name="pcm", bufs=2, space="PSUM"))

        ident = const_pool.tile([P, P], bf16)
        make_identity(nc, ident[:])
        onesf = const_pool.tile([P, 1], bf16)
        nc.vector.memset(onesf[:], 1.0)

        # double-banked Q^T/K^T/V_ext (rep parity) so rep r+1's
        # projections overlap rep r's last attention chunk
        QT, KT, VE4 = [], [], []
        for bk in range(nbank):
            QT.append([qt_pool.tile([P, S], bf16, name=f"qt{bk}_{m}",
                                    tag=f"qt{bk}_{m}")
                       for m in range(NMT)])
            KT.append([qt_pool.tile([P, S], bf16, name=f"kt{bk}_{m}",
                                    tag=f"kt{bk}_{m}")
                       for m in range(NMT)])
            # V_ext: per (head, key-tile) a [128, 65] stationary block;
            # col 64 stays 1.0 (projection copies only touch cols 0..63).
            VE = ve_pool.tile([P, HL * NKA * 65], bf16, name=f"ve{bk}",
                              tag=f"ve{bk}")
            nc.vector.tensor_copy(
                VE[:].rearrange("p (b c) -> p b c", c=65)[:, :, 64:65],
                onesf[:].broadcast_to([P, HL * NKA, 1]),
            )
            VE4.append(VE[:].rearrange("p (h ka c) -> p h ka c",
                                       h=HL, c=65))

        # single [128,128] lower-triangular keep-mask: tri[k,c]=1 iff c>=k
        tri = const_pool.tile([P, P], bf16)
        nc.vector.memset(tri[:], 1.0)
        nc.gpsimd.affine_select(
            out=tri[:],
            in_=tri[:],
            compare_op=ALU.is_ge,
            fill=0.0,
            base=0,
            pattern=[[1, P]],
            channel_multiplier=-1,
        )

        # resident weights, DMA'd in per-128-row slices from the otherwise
        # idle SP sequencer so Pool isn't a dispatch bottleneck.
        WS = {}
        for wname in ("v", "k", "q"):
            ws = ws_pool.tile([P, NKT * F], bf16, name=f"ws_{wname}",
                              tag=f"ws_{wname}")
            for kt in range(NKT):
                nc.sync.dma_start(
                    ws[:, kt * F : (kt + 1) * F],
                    w_by_name[wname][kt * P : (kt + 1) * P, :],
                )
            WS[wname] = ws
        WO = wo_pool.tile([P, NMT * D], bf16)
        for ft in range(NMT):
            nc.sync.dma_start(
                WO[:, ft * D : (ft + 1) * D],
                wo_d[ft * P : (ft + 1) * P, :],
            )

        def dma_xt(tch):
            xt_t = xt_pool.tile([P, NKT * CH], bf16)
            for kt in range(NKT):
                # Pool queue: parallel with SP's weight DMAs
                nc.gpsimd.dma_start(
                    xt_t[:, kt * CH : (kt + 1) * CH],
                    xt_d[kt * P : (kt + 1) * P,
                         tch * CH : (tch + 1) * CH],
                )
            return xt_t

        def proj_half(bk, tch, xt_t, wname, mt, half, state):
            # split per-mt projection into two ~850ns fill units sharing
            # one PSUM accumulator
            ws = WS[wname]
            k0 = half * (NKT // 2)
            if half == 0:
                state["pp"] = pp_pool.tile([P, CH], f32, name="pp",
                                           tag="pp")
            pp = state["pp"]
            for kt in range(k0, k0 + NKT // 2):
                nc.tensor.matmul(
                    pp[:],
                    ws[:, kt * F + mt * P : kt * F + (mt + 1) * P],
                    xt_t[:, kt * CH : (kt + 1) * CH],
                    start=(kt == 0),
                    stop=(kt == NKT - 1),
                )
            if half == 0:
                return
            if wname == "q":
                nc.vector.tensor_copy(
                    QT[bk][mt][:, tch * CH : (tch + 1) * CH], pp[:]
                )
            elif wname == "k":
                nc.vector.tensor_copy(
                    KT[bk][mt][:, tch * CH : (tch + 1) * CH], pp[:]
                )
            else:
                vs = vs_pool.tile([P, CH], bf16)
                nc.vector.tensor_copy(vs[:], pp[:])
                for j in range(CH // P):
                    ka = tch * (CH // P) + j
                    ptp = ps_s_pool.tile([P, P], bf16, tag="ps")
                    nc.tensor.transpose(
                        ptp[:], vs[:, j * P : (j + 1) * P], ident[:]
                    )
                    # both heads' 64-col halves in one copy
                    nc.vector.tensor_copy(
                        VE4[bk][:, 2 * mt : 2 * mt + 2, ka, 0:HD],
                        ptp[:].rearrange(
                            "p (hh c) -> p hh c", hh=2
                        )[:, :, 0:HD],
                    )

        def proj_groups(bk, tch, xt_t):
            out = []
            for w in ("v", "k", "q"):
                for m in range(NMT):
                    st = {}
                    for hf in range(2):
                        out.append(
                            lambda w=w, m=m, hf=hf, st=st:
                                proj_half(bk, tch, xt_t, w, m, hf, st)
                        )
            return out

        def wo_part(qc, ctc, tt4, ncol, state):
            if ncol == 0:
                state["osb"] = osb_pool.tile([P, D], f32, name="osb")
            osb = state["osb"]
            # pp pool, not pcm: pcq buffers are released by the (slower)
            # reciprocal+scale chain, pp buffers by a plain copy
            po = pp_pool.tile([P, CH], f32, name="po", tag="pp")
            for ft in range(NMT):
                nc.tensor.matmul(
                    po[:],
                    ctc[ft][:, tt4 * P : (tt4 + 1) * P],
                    WO[:, ft * D + ncol * CH : ft * D + (ncol + 1) * CH],
                    start=(ft == 0),
                    stop=(ft == NMT - 1),
                )
            # alternate engines: the po->osb copy gates the pp-pool WAR
            # for the next po, and DVE's in-order queue can back up ~3us
            if (tt4 + ncol) % 2 == 0:
                nc.scalar.copy(
                    osb[:, ncol * CH : (ncol + 1) * CH], po[:]
                )
            else:
                nc.vector.tensor_copy(
                    osb[:, ncol * CH : (ncol + 1) * CH], po[:]
                )
            if ncol == D // CH - 1:
                r0 = qc * CH + tt4 * P
                nc.gpsimd.dma_start(out_d[r0 : r0 + P, :], osb[:])

        def wo_groups(qc, ctc):
            out = []
            for t in range(CH // P):
                st = {}
                for ncol in range(D // CH):
                    out.append(
                        lambda t=t, ncol=ncol, st=st:
                            wo_part(qc, ctc, t, ncol, st)
                    )
            return out

        def attention_chunk(bk, qc, fill, carry_tpose=None):
            """Attention for query chunk qc reading bank bk, draining
            `fill` (list of emission callables) at key-pair granularity.
            `carry_tpose` is the previous chunk's last-head ctx transpose
            (emitted here, where its inputs are long ready). Returns
            (ctc, carry) where carry finishes this chunk's last head."""
            nka_q = 4 * qc + 4  # causal: key tiles 0..nka_q-1
            total_slots = HL * nka_q
            fill_state = [0, 0]  # [next fill idx, slot counter]

            def drain_fill():
                idx, slot = fill_state
                while (idx < len(fill)
                       and idx * total_slots <= slot * len(fill)):
                    fill[idx]()
                    idx += 1
                fill_state[0] = idx
                fill_state[1] = slot + 1

            ctc = [ctc_pool.tile([P, CH], bf16, name=f"ctc{m}",
                                 tag=f"ctc{m}")
                   for m in range(NMT)]

            def emit_norm_dve(pcq, ctq):
                # denominators live at col 64 of each qb block ->
                # per-partition scale, no PE broadcast needed.
                rec4 = rec_pool.tile([P, 4], bf16)
                pcq3 = pcq[:].rearrange("p (qb c) -> p qb c", c=HD + 1)
                rec3 = rec4[:].rearrange("p (b o) -> p b o", o=1)
                with nc.allow_low_precision(
                    reason="1/l rounded to bf16 scale"
                ):
                    nc.vector.reciprocal(rec3, pcq3[:, :, HD : HD + 1])
                nc.vector.tensor_mul(
                    ctq[:].rearrange("p (qb c) -> p qb c", c=HD),
                    pcq3[:, :, 0:HD],
                    rec3.broadcast_to([P, 4, HD]),
                )

            def emit_norm_tpose(ctq, mt, hrow):
                # [q, hd] -> [hd, q] for the f-major Wo projection
                for qb in range(4):
                    ptq = ps_s_pool.tile([HD, P], bf16, tag="ps")
                    nc.tensor.transpose(
                        ptq[:], ctq[:, qb * HD : (qb + 1) * HD], ident[:]
                    )
                    nc.vector.tensor_copy(
                        ctc[mt][hrow : hrow + HD, qb * P : (qb + 1) * P],
                        ptq[:],
                    )

            if carry_tpose is not None:
                carry_tpose()
            pending_tpose = None
            for h in range(HL):
                mt = h // 2
                hrow = (h % 2) * HD
                pcq = pcm_pool.tile([P, 4 * (HD + 1)], f32, tag="pcm")
                ctq = ctq_pool.tile([P, 4 * HD], bf16)
                ps1s = {}
                pt1s = {}

                def emit_scores(kt):
                    j = kt - 4 * qc  # diag block index if >= 0
                    lo = j * P if j >= 0 else 0
                    ps1 = ps_s_pool.tile([P, CH], f32, tag="ps")
                    pt1 = ptile_pool.tile([P, CH], bf16)
                    ps1s[kt] = ps1
                    pt1s[kt] = pt1
                    nc.tensor.matmul(
                        ps1[:, lo:],
                        KT[bk][mt][hrow : hrow + HD,
                                   kt * P : (kt + 1) * P],
                        QT[bk][mt][hrow : hrow + HD,
                                   qc * CH + lo : (qc + 1) * CH],
                        start=True,
                        stop=True,
                    )
                    # flipped ctx only reads q-blocks >= j, so the
                    # below-diagonal columns need no zeroing at all
                    nc.scalar.activation(
                        pt1[:, lo:], ps1[:, lo:], AF.Exp, scale=0.125
                    )
                    if j >= 0:
                        nc.vector.tensor_mul(
                            pt1[:, lo : lo + P],
                            pt1[:, lo : lo + P],
                            tri[:],
                        )

                def emit_ctx(kt):
                    pt1 = pt1s.pop(kt)
                    ps1s.pop(kt)
                    j = kt - 4 * qc
                    # masked (triangular) q-block last: its DVE mask
                    # finishes while the other blocks' ctx runs
                    if j >= 0:
                        qb_order = list(range(j + 1, 4)) + [j]
                    else:
                        qb_order = list(range(4))
                    for qb in qb_order:
                        # start=True clears has_written for the WHOLE
                        # bank, so only the very first matmul into the
                        # pcq bank may set it; later qb first-writes
                        # overwrite-where-clear per element.
                        nc.tensor.matmul(
                            pcq[:, qb * (HD + 1)
                                : qb * (HD + 1) + HD + 1],
                            pt1[:, qb * P : (qb + 1) * P],
                            VE4[bk][:, h, kt, :],
                            start=(kt == 0 and qb == qb_order[0]),
                            stop=(kt == min(nka_q - 1, 4 * qc + qb)),
                        )

                # software-pipelined emission with 3-tile lookahead:
                # scores/exp of tiles i+1..i+3 are in flight while ctx
                # of tile i runs, so PE streams while ScalarE drains;
                # the previous head's ctx transposes are emitted a few
                # tiles in.
                LA = 2
                for t in range(min(LA, nka_q)):
                    emit_scores(t)
                for i in range(nka_q):
                    if i + LA < nka_q:
                        emit_scores(i + LA)
                    # fill lands BEFORE this tile's ctx so the PE has
                    # queued work while the tile's exp drains
                    drain_fill()
                    emit_ctx(i)
                    if (i == min(2, nka_q - 1)
                            and pending_tpose is not None):
                        emit_norm_tpose(*pending_tpose)
                        pending_tpose = None
                # the DVE part runs now (frees pcq for the pool); only
                # the PE transposes are deferred.
                emit_norm_dve(pcq, ctq)
                pending_tpose = (ctq, mt, hrow)
            while fill_state[0] < len(fill):
                fill[fill_state[0]]()
                fill_state[0] += 1
            args = pending_tpose
            carry = lambda a=args: emit_norm_tpose(*a)
            return ctc, carry

        # ---- the flattened rep x chunk stream -------------------------
        seq = [(r, t) for r in range(reps) for t in range(NCH)]
        xt_t = dma_xt(0)
        for g in proj_groups(0, 0, xt_t):
            g()
        pending_wo = []
        carry = None
        for idx, (r, tch) in enumerate(seq):
            fill = list(pending_wo)
            pending_wo = []
            if idx + 1 < len(seq):
                nr, nt = seq[idx + 1]
                xt_t = dma_xt(nt)
                fill += proj_groups(nr % nbank, nt, xt_t)
            ctc, carry = attention_chunk(r % nbank, tch, fill, carry)
            pending_wo = wo_groups(tch, ctc)
        carry()
        for g in pending_wo:
            g()

    nc.compile()
    return nc


def _get_nc(reps=1):
    key = f"nc{reps}"
    if key not in _NC_CACHE:
        _NC_CACHE[key] = _build_nc(reps)
    return _NC_CACHE[key]


def _make_in_maps(inputs):
    x = np.asarray(inputs["x"], dtype=np.float32)
    W_q = np.asarray(inputs["W_q"], dtype=np.float32)
    W_k = np.asarray(inputs["W_k"], dtype=np.float32)
    W_v = np.asarray(inputs["W_v"], dtype=np.float32)
    W_o = np.asarray(inputs["W_o"], dtype=np.float32)
    in_maps = []
    for c in range(8):
        b = c % 4
        hh = c // 4
        cols = slice(hh * F, (hh + 1) * F)
        in_maps.append(
            {
                "xt": np.ascontiguousarray(x[b].T).astype(BF16),
                "wq": np.ascontiguousarray(W_q[:, cols]).astype(BF16),
                "wk": np.ascontiguousarray(W_k[:, cols]).astype(BF16),
                "wv": np.ascontiguousarray(W_v[:, cols]).astype(BF16),
                "wo": np.ascontiguousarray(W_o[cols, :]).astype(BF16),
            }
        )
    return in_maps


def kernel(x, W_q, W_k, W_v, W_o, b_o):
    from concourse.bass_utils import run_bass_kernel_spmd

    b_o = np.asarray(b_o, dtype=np.float32)
    nc = _get_nc()
    in_maps = _make_in_maps(
        {"x": x, "W_q": W_q, "W_k": W_k, "W_v": W_v, "W_o": W_o}
    )
    res = run_bass_kernel_spmd(nc, in_maps, core_ids=list(range(8)))

    full = np.empty((B, S, D), dtype=np.float32)
    for b in range(B):
        full[b] = res.results[b]["out"] + res.results[b + 4]["out"] + b_o
    return full


# revision 77
# speedup vs baseline: 1.5073x; 1.4480x over previous
"""Causal MHA kernel for 8 TRN2 NeuronCores.

Problem: x[4,2048,1024], 16 heads, hd=64, causal softmax attention, f32.

Sharding: core c handles batch c%4 and head-half c//4 (8 heads).
Each core computes its 8 heads' attention plus the row-slice of the
output projection; the host sums the two partials per batch (the
all-reduce of the row-parallel W_o split) and adds b_o.

v5: bf16 operands everywhere (PSUM accumulation stays f32; operand
quantization ~0.5% rel err, gate 2e-2). Weights are DMA'd once and
stay resident in SBUF.

Schedule: one software-pipelined stream over rep x token-chunk. Score
tiles are pipelined singly (one 128x512 tile per stage) with a 2-tile
lookahead through a 4-buffer PSUM pool — deeper and finer than pair
granularity, which only left ~44ns of per-slot slack against ScalarE's
exp cadence. During attention for query chunk t (paced by ScalarE exp
throughput), the PE work of the NEXT chunk's Q/K/V projections and the
PREVIOUS chunk's output projection is drained in at tile granularity,
so the in-order PE queue always has independent work while ScalarE
drains.
The chunk stream crosses rep boundaries; Q^T/K^T/V^T live in two
SBUF bank sets (rep parity) so the next rep's projections can overlap
the previous rep's last attention chunk without WAR stalls. ScalarE
runs ONLY exp; every PSUM->SBUF copy is on the Vector engine; memsets
and DMA dispatch are on GpSimd/SP.

Attention uses a flipped ctx product: probs tiles [keys, q] are the
stationary operand and V_ext [keys, hd+1] the moving one, producing
ctx in [q, hd] orientation with full PE output-column utilization
(half the streaming cycles of the [hd+1, q] orientation) and natural
causal skipping of all-masked 128-blocks. The trailing ones-column of
V_ext makes PSUM col 64 of each q-block the softmax denominator, so
normalization is a per-partition reciprocal + scale on DVE (no PE
broadcast). A PE transpose per q-block returns ctx to the f-major
layout the W_o projection needs. Within a head, score matmuls run
2 tiles ahead of the ctx matmuls; a head's ctx transposes are
deferred into the next head's stream (the last head's across the
chunk boundary). Diagonal 128x512 score tiles are
trimmed: exp only the valid column range and one [128,128] triangular
mask multiply; below-diagonal prob columns are never read at all in
the flipped ctx, so they need no zeroing.

Device-side layout: everything transposed. Host ships x[b].T so the
contraction dim (D) lands on SBUF partitions.
"""

import numpy as np
import ml_dtypes

BF16 = ml_dtypes.bfloat16

B, S, D, H, HD = 4, 2048, 1024, 16, 64
HL = 8            # heads per core
F = HL * HD       # 512 local head features
P = 128
CH = 512          # free-dim chunk for matmuls
NKT = D // P      # 8 contraction tiles for projections
NMT = F // P      # 4 head-pair tiles
NCH = S // CH     # 4 token chunks
NKA = S // P      # 16 attention key tiles

_NC_CACHE = {}


def _build_nc(reps=1):
    from contextlib import ExitStack

    import concourse.bass as bass
    import concourse.tile as tile
    from concourse import bacc, mybir
    from concourse.masks import make_identity

    f32 = mybir.dt.float32
    bf16 = mybir.dt.bfloat16
    AF = mybir.ActivationFunctionType
    ALU = mybir.AluOpType

    nc = bacc.Bacc("TRN2", target_bir_lowering=False)
    xt_d = nc.declare_dram_parameter("xt", [D, S], bf16, isOutput=False)
    wq_d = nc.declare_dram_parameter("wq", [D, F], bf16, isOutput=False)
    wk_d = nc.declare_dram_parameter("wk", [D, F], bf16, isOutput=False)
    wv_d = nc.declare_dram_parameter("wv", [D, F], bf16, isOutput=False)
    wo_d = nc.declare_dram_parameter("wo", [F, D], bf16, isOutput=False)
    out_d = nc.declare_dram_parameter("out", [S, D], f32, isOutput=True)
    w_by_name = {"q": wq_d, "k": wk_d, "v": wv_d}

    nbank = 2 if reps > 1 else 1

    with tile.TileContext(nc) as tc, ExitStack() as ctx:
        const_pool = ctx.enter_context(tc.tile_pool(name="const", bufs=1))
        qt_pool = ctx.enter_context(tc.tile_pool(name="qt", bufs=1))
        ve_pool = ctx.enter_context(tc.tile_pool(name="ve", bufs=1))
        wo_pool = ctx.enter_context(tc.tile_pool(name="wo", bufs=1))
        ws_pool = ctx.enter_context(tc.tile_pool(name="ws", bufs=1))
        xt_pool = ctx.enter_context(tc.tile_pool(name="xt", bufs=2))
        vs_pool = ctx.enter_context(tc.tile_pool(name="vstage", bufs=2))
        ptile_pool = ctx.enter_context(tc.tile_pool(name="ptile", bufs=6))
        ctc_pool = ctx.enter_context(tc.tile_pool(name="ctc", bufs=2))
        rec_pool = ctx.enter_context(tc.tile_pool(name="rec", bufs=2))
        ctq_pool = ctx.enter_context(tc.tile_pool(name="ctq", bufs=2))
        osb_pool = ctx.enter_context(tc.tile_pool(name="osb", bufs=2))
        pp_pool = ctx.enter_context(
            tc.tile_pool(name="pp", bufs=2, space="PSUM"))
        ps_s_pool = ctx.enter_context(
            tc.tile_pool(name="ps_s", bufs=4, space="PSUM"))
        pcm_pool = ctx.enter_context(
            tc.tile_pool(name="pcm", bufs=2, space="PSUM"))

        ident = const_pool.tile([P, P], bf16)
        make_identity(nc, ident[:])
        onesf = const_pool.tile([P, 1], bf16)
        nc.vector.memset(onesf[:], 1.0)

        # double-banked Q^T/K^T/V_ext (rep parity) so rep r+1's
        # projections overlap rep r's last attention chunk
        QT, KT, VE4 = [], [], []
        for bk in range(nbank):
            QT.append([qt_pool.tile([P, S], bf16, name=f"qt{bk}_{m}",
                                    tag=f"qt{bk}_{m}")
                       for m in range(NMT)])
            KT.append([qt_pool.tile([P, S], bf16, name=f"kt{bk}_{m}",
                                    tag=f"kt{bk}_{m}")
                       for m in range(NMT)])
            # V_ext: per (head, key-tile) a [128, 65] stationary block;
            # col 64 stays 1.0 (projection copies only touch cols 0..63).
            VE = ve_pool.tile([P, HL * NKA * 65], bf16, name=f"ve{bk}",
                              tag=f"ve{bk}")
            nc.vector.tensor_copy(
                VE[:].rearrange("p (b c) -> p b c", c=65)[:, :, 64:65],
                onesf[:].broadcast_to([P, HL * NKA, 1]),
            )
            VE4.append(VE[:].rearrange("p (h ka c) -> p h ka c",
                                       h=HL, c=65))

        # single [128,128] lower-triangular keep-mask: tri[k,c]=1 iff c>=k
        tri = const_pool.tile([P, P], bf16)
        nc.vector.memset(tri[:], 1.0)
        nc.gpsimd.affine_select(
            out=tri[:],
            in_=tri[:],
            compare_op=ALU.is_ge,
            fill=0.0,
            base=0,
            pattern=[[1, P]],
            channel_multiplier=-1,
        )

        # resident weights, DMA'd in per-128-row slices from the otherwise
        # idle SP sequencer so Pool isn't a dispatch bottleneck.
        WS = {}
        for wname in ("v", "k", "q"):
            ws = ws_pool.tile([P, NKT * F], bf16, name=f"ws_{wname}",
                              tag=f"ws_{wname}")
            for kt in range(NKT):
                nc.sync.dma_start(
                    ws[:, kt * F : (kt + 1) * F],
                    w_by_name[wname][kt * P : (kt + 1) * P, :],
                )
            WS[wname] = ws
        WO = wo_pool.tile([P, NMT * D], bf16)
        for ft in range(NMT):
            nc.sync.dma_start(
                WO[:, ft * D : (ft + 1) * D],
                wo_d[ft * P : (ft + 1) * P, :],
            )

        def dma_xt(tch):
            xt_t = xt_pool.tile([P, NKT * CH], bf16)
            for kt in range(NKT):
                # Pool queue: parallel with SP's weight DMAs
                nc.gpsimd.dma_start(
                    xt_t[:, kt * CH : (kt + 1) * CH],
                    xt_d[kt * P : (kt + 1) * P,
                         tch * CH : (tch + 1) * CH],
                )
            return xt_t

        def proj_half(bk, tch, xt_t, wname, mt, half, state):
            # split per-mt projection into two ~850ns fill units sharing
            # one PSUM accumulator
            ws = WS[wname]
            k0 = half * (NKT // 2)
            if half == 0:
                state["pp"] = pp_pool.tile([P, CH], f32, name="pp",
                                           tag="pp")
            pp = state["pp"]
            for kt in range(k0, k0 + NKT // 2):
                nc.tensor.matmul(
                    pp[:],
                    ws[:, kt * F + mt * P : kt * F + (mt + 1) * P],
                    xt_t[:, kt * CH : (kt + 1) * CH],
                    start=(kt == 0),
                    stop=(kt == NKT - 1),
                )
            if half == 0:
                return
            if wname == "q":
                nc.vector.tensor_copy(
                    QT[bk][mt][:, tch * CH : (tch + 1) * CH], pp[:]
                )
            elif wname == "k":
                nc.vector.tensor_copy(
                    KT[bk][mt][:, tch * CH : (tch + 1) * CH], pp[:]
                )
            else:
                vs = vs_pool.tile([P, CH], bf16)
                nc.vector.tensor_copy(vs[:], pp[:])
                for j in range(CH // P):
                    ka = tch * (CH // P) + j
                    ptp = ps_s_pool.tile([P, P], bf16, tag="ps")
                    nc.tensor.transpose(
                        ptp[:], vs[:, j * P : (j + 1) * P], ident[:]
                    )
                    # both heads' 64-col halves in one copy
                    nc.vector.tensor_copy(
                        VE4[bk][:, 2 * mt : 2 * mt + 2, ka, 0:HD],
                        ptp[:].rearrange(
                            "p (hh c) -> p hh c", hh=2
                        )[:, :, 0:HD],
                    )

        def proj_groups(bk, tch, xt_t):
            out = []
            for w in ("v", "k", "q"):
                for m in range(NMT):
                    st = {}
                    for hf in range(2):
                        out.append(
                            lambda w=w, m=m, hf=hf, st=st:
                                proj_half(bk, tch, xt_t, w, m, hf, st)
                        )
            return out

        def wo_part(qc, ctc, tt4, ncol, state):
            if ncol == 0:
                state["osb"] = osb_pool.tile([P, D], f32, name="osb")
            osb = state["osb"]
            # pp pool, not pcm: pcq buffers are released by the (slower)
            # reciprocal+scale chain, pp buffers by a plain copy
            po = pp_pool.tile([P, CH], f32, name="po", tag="pp")
            for ft in range(NMT):
                nc.tensor.matmul(
                    po[:],
                    ctc[ft][:, tt4 * P : (tt4 + 1) * P],
                    WO[:, ft * D + ncol * CH : ft * D + (ncol + 1) * CH],
                    start=(ft == 0),
                    stop=(ft == NMT - 1),
                )
            # alternate engines: the po->osb copy gates the pp-pool WAR
            # for the next po, and DVE's in-order queue can back up ~3us
            if (tt4 + ncol) % 2 == 0:
                nc.scalar.copy(
                    osb[:, ncol * CH : (ncol + 1) * CH], po[:]
                )
            else:
                nc.vector.tensor_copy(
                    osb[:, ncol * CH : (ncol + 1) * CH], po[:]
                )
            if ncol == D // CH - 1:
                r0 = qc * CH + tt4 * P
                nc.gpsimd.dma_start(out_d[r0 : r0 + P, :], osb[:])

        def wo_groups(qc, ctc):
            out = []
            for t in range(CH // P):
                st = {}
                for ncol in range(D // CH):
                    out.append(
                        lambda t=t, ncol=ncol, st=st:
                            wo_part(qc, ctc, t, ncol, st)
                    )
            return out

        def attention_chunk(bk, qc, fill, carry_tpose=None):
            """Attention for query chunk qc reading bank bk, draining
            `fill` (list of emission callables) at key-pair granularity.
            `carry_tpose` is the previous chunk's last-head ctx transpose
            (emitted here, where its inputs are long ready). Returns
            (ctc, carry) where carry finishes this chunk's last head."""
            nka_q = 4 * qc + 4  # causal: key tiles 0..nka_q-1
            total_slots = HL * nka_q
            fill_state = [0, 0]  # [next fill idx, slot counter]

            def drain_fill():
                idx, slot = fill_state
                while (idx < len(fill)
                       and idx * total_slots <= slot * len(fill)):
                    fill[idx]()
                    idx += 1
                fill_state[0] = idx
                fill_state[1] = slot + 1

            ctc = [ctc_pool.tile([P, CH], bf16, name=f"ctc{m}",
                                 tag=f"ctc{m}")
                   for m in range(NMT)]

            def emit_norm_dve(pcq, ctq):
                # denominators live at col 64 of each qb block ->
                # per-partition scale, no PE broadcast needed.
                rec4 = rec_pool.tile([P, 4], bf16)
                pcq3 = pcq[:].rearrange("p (qb c) -> p qb c", c=HD + 1)
                rec3 = rec4[:].rearrange("p (b o) -> p b o", o=1)
                with nc.allow_low_precision(
                    reason="1/l rounded to bf16 scale"
                ):
                    nc.vector.reciprocal(rec3, pcq3[:, :, HD : HD + 1])
                nc.vector.tensor_mul(
                    ctq[:].rearrange("p (qb c) -> p qb c", c=HD),
                    pcq3[:, :, 0:HD],
                    rec3.broadcast_to([P, 4, HD]),
                )

            def emit_norm_tpose(ctq, mt, hrow):
                # [q, hd] -> [hd, q] for the f-major Wo projection
                for qb in range(4):
                    ptq = ps_s_pool.tile([HD, P], bf16, tag="ps")
                    nc.tensor.transpose(
                        ptq[:], ctq[:, qb * HD : (qb + 1) * HD], ident[:]
                    )
                    nc.vector.tensor_copy(
                        ctc[mt][hrow : hrow + HD, qb * P : (qb + 1) * P],
                        ptq[:],
                    )

            if carry_tpose is not None:
                carry_tpose()
            pending_tpose = None
            for h in range(HL):
                mt = h // 2
                hrow = (h % 2) * HD
                pcq = pcm_pool.tile([P, 4 * (HD + 1)], f32, tag="pcm")
                ctq = ctq_pool.tile([P, 4 * HD], bf16)
                ps1s = {}
                pt1s = {}

                def emit_scores(kt):
                    j = kt - 4 * qc  # diag block index if >= 0
                    lo = j * P if j >= 0 else 0
                    ps1 = ps_s_pool.tile([P, CH], f32, tag="ps")
                    pt1 = ptile_pool.tile([P, CH], bf16)
                    ps1s[kt] = ps1
                    pt1s[kt] = pt1
                    nc.tensor.matmul(
                        ps1[:, lo:],
                        KT[bk][mt][hrow : hrow + HD,
                                   kt * P : (kt + 1) * P],
                        QT[bk][mt][hrow : hrow + HD,
                                   qc * CH + lo : (qc + 1) * CH],
                        start=True,
                        stop=True,
                    )
                    # flipped ctx only reads q-blocks >= j, so the
                    # below-diagonal columns need no zeroing at all
                    nc.scalar.activation(
                        pt1[:, lo:], ps1[:, lo:], AF.Exp, scale=0.125
                    )
                    if j >= 0:
                        nc.vector.tensor_mul(
                            pt1[:, lo : lo + P],
                            pt1[:, lo : lo + P],
                            tri[:],
                        )

                def emit_ctx(kt):
                    pt1 = pt1s.pop(kt)
                    ps1s.pop(kt)
                    j = kt - 4 * qc
                    # masked (triangular) q-block last: its DVE mask
                    # finishes while the other blocks' ctx runs
                    if j >= 0:
                        qb_order = list(range(j + 1, 4)) + [j]
                    else:
                        qb_order = list(range(4))
                    for qb in qb_order:
                        # start=True clears has_written for the WHOLE
                        # bank, so only the very first matmul into the
                        # pcq bank may set it; later qb first-writes
                        # overwrite-where-clear per element.
                        nc.tensor.matmul(
                            pcq[:, qb * (HD + 1)
                                : qb * (HD + 1) + HD + 1],
                            pt1[:, qb * P : (qb + 1) * P],
                            VE4[bk][:, h, kt, :],
                            start=(kt == 0 and qb == qb_order[0]),
                            stop=(kt == min(nka_q - 1, 4 * qc + qb)),
                        )

                # software-pipelined emission with 3-tile lookahead:
                # scores/exp of tiles i+1..i+3 are in flight while ctx
                # of tile i runs, so PE streams while ScalarE drains;
                # the previous head's ctx transposes are emitted a few
                # tiles in.
                LA = 2
                for t in range(min(LA, nka_q)):
                    emit_scores(t)
                for i in range(nka_q):
                    if i + LA < nka_q:
                        emit_scores(i + LA)
                    # fill lands BEFORE this tile's ctx so the PE has
                    # queued work while the tile's exp drains
                    drain_fill()
                    emit_ctx(i)
                    if (i == min(2, nka_q - 1)
                            and pending_tpose is not None):
                        emit_norm_tpose(*pending_tpose)
                        pending_tpose = None
                # the DVE part runs now (frees pcq for the pool); only
                # the PE transposes are deferred.
                emit_norm_dve(pcq, ctq)
                pending_tpose = (ctq, mt, hrow)
            while fill_state[0] < len(fill):
                fill[fill_state[0]]()
                fill_state[0] += 1
            args = pending_tpose
            carry = lambda a=args: emit_norm_tpose(*a)
            return ctc, carry

        # ---- the flattened rep x chunk stream -------------------------
        seq = [(r, t) for r in range(reps) for t in range(NCH)]
        xt_t = dma_xt(0)
        for g in proj_groups(0, 0, xt_t):
            g()
        pending_wo = []
        carry = None
        for idx, (r, tch) in enumerate(seq):
            fill = list(pending_wo)
            pending_wo = []
            if idx + 1 < len(seq):
                nr, nt = seq[idx + 1]
                xt_t = dma_xt(nt)
                fill += proj_groups(nr % nbank, nt, xt_t)
            ctc, carry = attention_chunk(r % nbank, tch, fill, carry)
            pending_wo = wo_groups(tch, ctc)
        carry()
        for g in pending_wo:
            g()

    nc.compile()
    return nc


def _get_nc(reps=1):
    key = f"nc{reps}"
    if key not in _NC_CACHE:
        _NC_CACHE[key] = _build_nc(reps)
    return _NC_CACHE[key]


def _make_in_maps(inputs):
    x = np.asarray(inputs["x"], dtype=np.float32)
    W_q = np.asarray(inputs["W_q"], dtype=np.float32)
    W_k = np.asarray(inputs["W_k"], dtype=np.float32)
    W_v = np.asarray(inputs["W_v"], dtype=np.float32)
    W_o = np.asarray(inputs["W_o"], dtype=np.float32)
    in_maps = []
    for c in range(8):
        b = c % 4
        hh = c // 4
        cols = slice(hh * F, (hh + 1) * F)
        in_maps.append(
            {
                "xt": np.ascontiguousarray(x[b].T).astype(BF16),
                "wq": np.ascontiguousarray(W_q[:, cols]).astype(BF16),
                "wk": np.ascontiguousarray(W_k[:, cols]).astype(BF16),
                "wv": np.ascontiguousarray(W_v[:, cols]).astype(BF16),
                "wo": np.ascontiguousarray(W_o[cols, :]).astype(BF16),
            }
        )
    return in_maps


def kernel(x, W_q, W_k, W_v, W_o, b_o):
    from concourse.bass_utils import run_bass_kernel_spmd

    b_o = np.asarray(b_o, dtype=np.float32)
    nc = _get_nc()
    in_maps = _make_in_maps(
        {"x": x, "W_q": W_q, "W_k": W_k, "W_v": W_v, "W_o": W_o}
    )
    res = run_bass_kernel_spmd(nc, in_maps, core_ids=list(range(8)))

    full = np.empty((B, S, D), dtype=np.float32)
    for b in range(B):
        full[b] = res.results[b]["out"] + res.results[b + 4]["out"] + b_o
    return full
